# revision 1
# baseline (speedup 1.0000x reference)
"""nn_AttnDecoder TRN2 kernel: 8-core data-parallel (batch-sharded) Bass kernel.

Self-contained: builds/compiles the bass program on first call (cached),
shards inputs across 8 NeuronCores, runs via PJRT, reassembles full outputs.
"""
import numpy as np
import ml_dtypes
from contextlib import ExitStack

import jax
from jax.sharding import Mesh, PartitionSpec
from jax.experimental.shard_map import shard_map

import concourse.bass as bass
import concourse.tile as tile
from concourse import bacc, mybir
from concourse.bass2jax import (_bass_exec_p, install_neuronx_cc_hook,
                                partition_id_tensor)

F32 = mybir.dt.float32
F16 = mybir.dt.float16
BF16 = mybir.dt.bfloat16
AF = mybir.ActivationFunctionType
OP = mybir.AluOpType
BF = ml_dtypes.bfloat16

B, H, L, T, V, LAYERS = 512, 256, 256, 64, 128, 2
NB = 64
N_CORES = 8


def _build_nc(Tn=T, nb=NB, n_devices=N_CORES):
    nc = bacc.Bacc("TRN2", target_bir_lowering=False, debug=False,
                   num_devices=n_devices)
    TB = Tn * nb

    d_x = nc.dram_tensor("x_r", [nb, 2, 128, L], F32, kind="ExternalInput").ap()
    d_c1t = nc.dram_tensor("c1t", [128, 2 * H], BF16, kind="ExternalInput").ap()
    d_a2t = nc.dram_tensor("a2t", [128, 4 * 128], BF16, kind="ExternalInput").ap()
    d_ea1 = nc.dram_tensor("ea1", [128, 2 * TB], BF16, kind="ExternalInput").ap()
    d_c2e = nc.dram_tensor("c2e", [128, 2 * TB], BF16, kind="ExternalInput").ap()
    d_wrz = nc.dram_tensor("wrz", [128, LAYERS * 16 * 128], BF16, kind="ExternalInput").ap()
    d_win = nc.dram_tensor("win", [128, LAYERS * 4 * 128], BF16, kind="ExternalInput").ap()
    d_whn = nc.dram_tensor("whn", [128, LAYERS * 4 * 128], BF16, kind="ExternalInput").ap()
    d_owt = nc.dram_tensor("owt", [128, 2 * V], BF16, kind="ExternalInput").ap()
    d_outb = nc.dram_tensor("outb", [128, 1], F32, kind="ExternalInput").ap()
    d_onescol = nc.dram_tensor("onescol", [128, 1], BF16, kind="ExternalInput").ap()
    d_onesrow = nc.dram_tensor("onesrow", [1, 128], BF16, kind="ExternalInput").ap()

    d_logp = nc.dram_tensor("out_logp", [V, TB], F32, kind="ExternalOutput").ap()
    d_attn = nc.dram_tensor("out_attn", [2 * 128, TB], F32, kind="ExternalOutput").ap()

    with tile.TileContext(nc) as tc, ExitStack() as ctx:
        st = ctx.enter_context(tc.tile_pool(name="statics", bufs=1))
        work = ctx.enter_context(tc.tile_pool(name="work", bufs=2))
        xs = ctx.enter_context(tc.tile_pool(name="xs", bufs=3))
        psst = ctx.enter_context(tc.tile_pool(name="psst", bufs=1, space="PSUM"))

        PB = []
        for i in range(8):
            pb = psst.tile([128, 512], F32, tag=f"pbank{i}", name=f"pbank{i}")
            PB.append(pb)
        (PB_SC, PB_CP, PB_RZ0, PB_RZ1, PB_S1, PB_HN, PB_BZ, PB_LG) = PB

        c1t = st.tile([128, 2 * H], BF16)
        a2t = st.tile([128, 4 * 128], BF16)
        ea1 = st.tile([128, 2 * TB], BF16)
        c2e = st.tile([128, 2 * TB], BF16)
        wrz = st.tile([128, LAYERS * 16 * 128], BF16)
        win = st.tile([128, LAYERS * 4 * 128], BF16)
        whn = st.tile([128, LAYERS * 4 * 128], BF16)
        owt = st.tile([128, 2 * V], BF16)
        outb = st.tile([128, 1], F32)
        onescol = st.tile([128, 1], BF16)
        onesrow = st.tile([1, 128], BF16)
        onesrow16 = st.tile([1, 128], F16)
        for ap_d, ap_s in ((d_c1t, c1t), (d_a2t, a2t), (d_ea1, ea1), (d_c2e, c2e),
                           (d_wrz, wrz), (d_win, win), (d_whn, whn), (d_owt, owt),
                           (d_outb, outb), (d_onescol, onescol), (d_onesrow, onesrow)):
            nc.sync.dma_start(ap_s[:], ap_d[:])
        nc.vector.tensor_copy(onesrow16[:], onesrow[:])

        mbt = st.tile([128, nb * 2 * H], BF16)
        h1h = st.tile([128, (Tn + 1) * 2 * nb], BF16)
        exph = st.tile([128, 2 * TB], BF16)
        rzh = st.tile([1, TB], BF16)
        h0s = st.tile([128, 2 * 2 * nb], BF16)
        h0m = st.tile([128, 2 * 2 * nb], F32)
        h1m = st.tile([128, 2 * 2 * nb], F32)
        nc.vector.memset(h0s[:], 0.0)
        nc.vector.memset(h0m[:], 0.0)
        nc.vector.memset(h1m[:], 0.0)
        nc.vector.memset(h1h[:, 0:2 * nb], 0.0)

        with nc.named_scope("mbt"):
            for b in range(nb):
                x32 = xs.tile([128, 2 * L], F32, tag="x32", name="x32")
                x16 = xs.tile([128, 2 * L], BF16, tag="x16", name="x16")
                for hc in range(2):
                    nc.sync.dma_start(x32[:, hc * L:(hc + 1) * L], d_x[b, hc])
                nc.vector.tensor_copy(x16[:], x32[:])
                for lc2 in range(2):
                    mps = PB_LG[:, (b % 2) * 256:(b % 2) * 256 + H]
                    for hc in range(2):
                        nc.tensor.matmul(
                            mps,
                            x16[:, hc * L + lc2 * 128: hc * L + lc2 * 128 + 128],
                            c1t[:, hc * H:(hc + 1) * H],
                            start=(hc == 0), stop=(hc == 1))
                    dst = mbt[:, (b * 2 + lc2) * H:(b * 2 + lc2 + 1) * H]
                    if lc2 == 0:
                        nc.vector.tensor_copy(dst, mps)
                    else:
                        nc.scalar.copy(dst, mps)

        for t in range(Tn):
            pp, qq = t % 2, 1 - (t % 2)
            h0sh_prev = h0s[:, pp * 2 * nb:(pp + 1) * 2 * nb]
            h1sh_prev = h1h[:, t * 2 * nb:(t + 1) * 2 * nb]
            with nc.named_scope(f"s{t}"):
                sc = PB_SC[:, pp * 128: pp * 128 + 128]
                for lc in range(2):
                    for hc in range(2):
                        nc.tensor.matmul(
                            sc[:, lc * nb:(lc + 1) * nb],
                            a2t[:, (hc * 2 + lc) * 128:(hc * 2 + lc + 1) * 128],
                            h0sh_prev[:, hc * nb:(hc + 1) * nb],
                            start=(hc == 0), stop=(hc == 1))
                es = work.tile([128, 2 * nb], BF16, tag="es", name="es")
                nc.scalar.activation(es[:], sc, AF.Exp)
                eslot = exph[:, t * 2 * nb:(t + 1) * 2 * nb]
                nc.vector.tensor_tensor(eslot, es[:],
                                        ea1[:, t * 2 * nb:(t + 1) * 2 * nb], OP.mult)
                zrow = PB_BZ[0:1, pp * 256: pp * 256 + nb]
                for lc in range(2):
                    nc.tensor.matmul(zrow, onescol[:, 0:1],
                                     exph[:, (t * 2 + lc) * nb:(t * 2 + lc + 1) * nb],
                                     start=(lc == 0), stop=(lc == 1))
                rz16 = rzh[0:1, t * nb:(t + 1) * nb]
                with nc.allow_low_precision(reason="recipZ bf16 is plenty"):
                    nc.vector.reciprocal(rz16, zrow)
                cp = PB_CP[:, pp * 128: pp * 128 + 128]
                for b in range(nb):
                    for hc2 in range(2):
                        for lc in range(2):
                            nc.tensor.matmul(
                                cp[:, hc2 * nb + b: hc2 * nb + b + 1],
                                mbt[:, (b * 2 + lc) * H + hc2 * 128:
                                       (b * 2 + lc) * H + hc2 * 128 + 128],
                                exph[:, (t * 2 + lc) * nb + b:
                                        (t * 2 + lc) * nb + b + 1],
                                start=(lc == 0), stop=(lc == 1))
                bcp = PB_BZ[:, pp * 256 + 64: pp * 256 + 128]
                nc.tensor.matmul(bcp, onesrow[:, :], rz16, start=True, stop=True)
                bcs = work.tile([128, nb], F32, tag="bcs", name="bcs")
                nc.scalar.copy(bcs[:], bcp)
                t1 = work.tile([128, 2 * nb], BF16, tag="t1", name="t1")
                a_big, a_small = bass.broadcast_tensor_aps(
                    cp.rearrange("p (c b) -> p c b", c=2),
                    bcs[:].rearrange("p (c b) -> p c b", c=1))
                nc.vector.tensor_tensor(
                    t1[:].rearrange("p (c b) -> p c b", c=2), a_big, a_small, OP.mult)
                t2 = work.tile([128, 2 * nb], BF16, tag="t2", name="t2")
                nc.vector.tensor_tensor(t2[:], t1[:],
                                        c2e[:, t * 2 * nb:(t + 1) * 2 * nb], OP.add)
                g16 = work.tile([128, 2 * nb], BF16, tag="g16", name="g16")
                nc.vector.tensor_scalar_max(g16[:], t2[:], 0.0)

                xin = g16[:]
                for layer in range(LAYERS):
                    hm = h0m if layer == 0 else h1m
                    hsh_prev = h0sh_prev if layer == 0 else h1sh_prev
                    hm_prev = hm[:, pp * 2 * nb:(pp + 1) * 2 * nb]
                    rzp = (PB_RZ0 if layer == 0 else PB_RZ1)[:, pp * 256: pp * 256 + 256]
                    for mc in range(4):
                        for kc in range(4):
                            rhs = (xin[:, (kc % 2) * nb:(kc % 2 + 1) * nb] if kc < 2
                                   else hsh_prev[:, (kc - 2) * nb:(kc - 1) * nb])
                            nc.tensor.matmul(
                                rzp[:, mc * nb:(mc + 1) * nb],
                                wrz[:, (layer * 16 + kc * 4 + mc) * 128:
                                       (layer * 16 + kc * 4 + mc + 1) * 128],
                                rhs, start=(kc == 0), stop=(kc == 3))
                    s1p = PB_S1[:, (pp * 2 + layer) * 128:(pp * 2 + layer) * 128 + 128]
                    hnp = PB_HN[:, (pp * 2 + layer) * 128:(pp * 2 + layer) * 128 + 128]
                    for mc in range(2):
                        for kc in range(2):
                            nc.tensor.matmul(
                                s1p[:, mc * nb:(mc + 1) * nb],
                                win[:, (layer * 4 + kc * 2 + mc) * 128:
                                       (layer * 4 + kc * 2 + mc + 1) * 128],
                                xin[:, kc * nb:(kc + 1) * nb],
                                start=(kc == 0), stop=(kc == 1))
                            nc.tensor.matmul(
                                hnp[:, mc * nb:(mc + 1) * nb],
                                whn[:, (layer * 4 + kc * 2 + mc) * 128:
                                       (layer * 4 + kc * 2 + mc + 1) * 128],
                                hsh_prev[:, kc * nb:(kc + 1) * nb],
                                start=(kc == 0), stop=(kc == 1))
                    trz = work.tile([128, 4 * nb], BF16, tag=f"trz{layer}", name="trz")
                    nc.scalar.activation(trz[:], rzp, AF.Tanh, scale=0.5)
                    u2 = work.tile([128, 2 * nb], F32, tag=f"u2{layer}", name="u2")
                    nc.vector.scalar_tensor_tensor(u2[:], trz[:, 0:2 * nb], 1.0, hnp,
                                                   OP.add, OP.mult)
                    vv = work.tile([128, 2 * nb], F32, tag=f"vv{layer}", name="vv")
                    nc.vector.tensor_tensor(vv[:], s1p, u2[:], OP.add)
                    nn_t = work.tile([128, 2 * nb], BF16, tag=f"nn{layer}", name="nn")
                    nc.scalar.activation(nn_t[:], vv[:], AF.Tanh)
                    q_t = work.tile([128, 2 * nb], BF16, tag=f"q{layer}", name="q")
                    nc.vector.tensor_tensor(q_t[:], hsh_prev, nn_t[:], OP.subtract)
                    p_t = work.tile([128, 2 * nb], F32, tag=f"p{layer}", name="p")
                    nc.vector.tensor_tensor(p_t[:], hm_prev, nn_t[:], OP.add)
                    r2 = work.tile([128, 2 * nb], BF16, tag=f"r2{layer}", name="r2")
                    nc.vector.tensor_tensor(r2[:], q_t[:], trz[:, 2 * nb:4 * nb], OP.mult)
                    tt = work.tile([128, 2 * nb], F32, tag=f"tt{layer}", name="tt")
                    nc.vector.tensor_tensor(tt[:], p_t[:], r2[:], OP.add)
                    hm_new = hm[:, qq * 2 * nb:(qq + 1) * 2 * nb]
                    nc.vector.tensor_scalar_mul(hm_new, tt[:], 0.5)
                    if layer == 0:
                        sh_new = h0s[:, qq * 2 * nb:(qq + 1) * 2 * nb]
                    else:
                        sh_new = h1h[:, (t + 1) * 2 * nb:(t + 2) * 2 * nb]
                    nc.vector.tensor_scalar_mul(sh_new, tt[:], 0.5)
                    xin = sh_new

        NCH = min(512, TB)
        h1h4 = h1h[:].rearrange("p (t c b) -> p t c b", c=2, b=nb)
        with nc.named_scope("logits"):
            for ch in range(TB // NCH):
                tpc = NCH // nb
                lp = (PB_LG if ch % 2 == 0 else PB_CP)[:, 0:NCH]
                for kc in range(2):
                    rhs = h1h4[:, 1 + ch * tpc: 1 + (ch + 1) * tpc, kc, :]
                    nc.tensor.matmul(lp, owt[:, kc * V:(kc + 1) * V], rhs,
                                     start=(kc == 0), stop=(kc == 1))
                ex16 = work.tile([128, NCH], BF16, tag="ex16", name="ex16")
                nc.scalar.activation(ex16[:], lp, AF.Exp, bias=outb[:, 0:1])
                zr = (PB_SC if ch % 2 == 0 else PB_RZ0)[0:1, 0:NCH]
                nc.tensor.matmul(zr, onescol[:, 0:1], ex16[:], start=True, stop=True)
                lnz = work.tile([1, NCH], F16, tag="lnz", name="lnz")
                nc.scalar.activation(lnz[:], zr, AF.Ln)
                bcl = (PB_S1 if ch % 2 == 0 else PB_HN)[:, 0:NCH]
                nc.tensor.matmul(bcl, onesrow16[:, :], lnz[:], start=True, stop=True)
                bcl_s = work.tile([128, NCH], F16, tag="bcl_s", name="bcl_s")
                nc.scalar.copy(bcl_s[:], bcl)
                lout = work.tile([128, NCH], F32, tag="lout", name="lout")
                nc.vector.scalar_tensor_tensor(lout[:], lp, outb[:, 0:1], bcl_s[:],
                                               OP.add, OP.subtract)
                nc.sync.dma_start(d_logp[:, ch * NCH:(ch + 1) * NCH], lout[:])

        exph4 = exph[:].rearrange("p (t c b) -> p t c b", c=2, b=nb)
        with nc.named_scope("attn"):
            for ch in range(TB // NCH):
                tpc = NCH // nb
                bca = (PB_RZ1 if ch % 2 == 0 else PB_BZ)[:, 0:NCH]
                nc.tensor.matmul(bca, onesrow[:, :],
                                 rzh[0:1, ch * NCH:(ch + 1) * NCH],
                                 start=True, stop=True)
                for lc in range(2):
                    an = work.tile([128, NCH], F32, tag="an", name="an")
                    nc.vector.tensor_tensor(
                        an[:], exph4[:, ch * tpc:(ch + 1) * tpc, lc, :], bca, OP.mult)
                    nc.sync.dma_start(
                        d_attn[lc * 128:(lc + 1) * 128, ch * NCH:(ch + 1) * NCH], an[:])

    nc.compile()
    return nc


def _host_prep(inputs, core, Tn=T, nb=NB):
    x = np.asarray(inputs["x"])[:, :, 0, :]
    y = np.asarray(inputs["y"]).astype(np.int64)
    emb = np.asarray(inputs["emb"], np.float32)
    attn_w = np.asarray(inputs["attn_w"], np.float32)
    attn_b = np.asarray(inputs["attn_b"], np.float32)
    comb_w = np.asarray(inputs["comb_w"], np.float32)
    comb_b = np.asarray(inputs["comb_b"], np.float32)
    gwih = np.asarray(inputs["gru_w_ih"], np.float32)
    gwhh = np.asarray(inputs["gru_w_hh"], np.float32)
    gbih = np.asarray(inputs["gru_b_ih"], np.float32)
    gbhh = np.asarray(inputs["gru_b_hh"], np.float32)
    out_w = np.asarray(inputs["out_w"], np.float32)
    out_b = np.asarray(inputs["out_b"], np.float32)
    assert np.all(gbih == 0) and np.all(gbhh == 0), "nonzero GRU biases unsupported"

    sl = slice(core * nb, (core + 1) * nb)
    tokens = np.concatenate([np.zeros((B, 1), y.dtype), y[:, :-1]], axis=1)[sl]
    xb = x[sl]
    A1, A2 = attn_w[:, :H], attn_w[:, H:]
    C1, C2 = comb_w[:, :H], comb_w[:, H:]

    ea1_tab = np.exp(A1 @ emb.T + attn_b[:, None]).astype(np.float32)
    c2e_tab = (C2 @ emb.T + comb_b[:, None]).astype(np.float32)

    tb = Tn * nb
    ea1_g = ea1_tab[:, tokens.T[:Tn]]
    ea1_r = ea1_g.reshape(2, 128, Tn, nb).transpose(1, 2, 0, 3).reshape(128, 2 * tb)
    c2e_g = c2e_tab[:, tokens.T[:Tn]]
    c2e_r = c2e_g.reshape(2, 128, Tn, nb).transpose(1, 2, 0, 3).reshape(128, 2 * tb)

    def lhs_blocks(Wt, n_kc, n_mc):
        K, M = Wt.shape
        assert K == n_kc * 128 and M == n_mc * 128
        a = Wt.reshape(n_kc, 128, n_mc, 128).transpose(1, 0, 2, 3)
        return a.reshape(128, n_kc * n_mc * 128)

    c1t_rhs = C1.T.reshape(2, 128, 256).transpose(1, 0, 2).reshape(128, 2 * H)
    a2t_r = lhs_blocks(A2.T, 2, 2)

    wrz_list, win_list, whn_list = [], [], []
    for lyr in range(LAYERS):
        Wi, Wh = gwih[lyr], gwhh[lyr]
        lhs_rz = np.vstack([Wi[0:2 * H, :].T, Wh[0:2 * H, :].T])
        wrz_list.append(lhs_blocks(lhs_rz, 4, 4))
        win_list.append(lhs_blocks(Wi[2 * H:3 * H, :].T, 2, 2))
        whn_list.append(lhs_blocks(0.5 * Wh[2 * H:3 * H, :].T, 2, 2))
    wrz_r = np.concatenate(wrz_list, axis=1)
    win_r = np.concatenate(win_list, axis=1)
    whn_r = np.concatenate(whn_list, axis=1)
    owt_r = lhs_blocks(out_w.T, 2, 1)

    return {
        "x_r": np.ascontiguousarray(xb.reshape(nb, 2, 128, L), np.float32),
        "c1t": c1t_rhs.astype(BF),
        "a2t": a2t_r.astype(BF),
        "ea1": ea1_r.astype(BF),
        "c2e": c2e_r.astype(BF),
        "wrz": wrz_r.astype(BF),
        "win": win_r.astype(BF),
        "whn": whn_r.astype(BF),
        "owt": owt_r.astype(BF),
        "outb": out_b.reshape(128, 1).astype(np.float32),
        "onescol": np.ones((128, 1), BF),
        "onesrow": np.ones((1, 128), BF),
    }


def _host_post(res, Tn=T, nb=NB):
    lp = res["out_logp"].reshape(V, Tn, nb)
    outs = np.ascontiguousarray(lp.transpose(2, 0, 1))
    at = res["out_attn"].reshape(2, 128, Tn, nb)
    attns = np.ascontiguousarray(at.transpose(3, 2, 0, 1).reshape(nb, Tn, L))
    return outs, attns


class _Runner:
    def __init__(self, nc, n_cores):
        install_neuronx_cc_hook()
        self.nc = nc
        self.n_cores = n_cores
        in_names, out_names, out_avals, zero_outs = [], [], [], []
        partition_name = nc.partition_id_tensor.name if nc.partition_id_tensor else None
        for alloc in nc.m.functions[0].allocations:
            if not isinstance(alloc, mybir.MemoryLocationSet):
                continue
            name = alloc.memorylocations[0].name
            if alloc.kind == "ExternalInput":
                if name != partition_name:
                    in_names.append(name)
            elif alloc.kind == "ExternalOutput":
                shape = tuple(alloc.tensor_shape)
                dtype = mybir.dt.np(alloc.dtype)
                out_names.append(name)
                out_avals.append(jax.core.ShapedArray(shape, dtype))
                zero_outs.append(np.zeros(shape, dtype))
        self.in_names, self.out_names = in_names, out_names
        self.out_avals, self.zero_outs = out_avals, zero_outs
        n_params, n_outs = len(in_names), len(out_avals)
        all_in_names = list(in_names) + list(out_names)
        if partition_name is not None:
            all_in_names.append(partition_name)

        def _body(*args):
            operands = list(args)
            if partition_name is not None:
                operands.append(partition_id_tensor())
            outs = _bass_exec_p.bind(
                *operands,
                out_avals=tuple(out_avals),
                in_names=tuple(all_in_names),
                out_names=tuple(out_names),
                lowering_input_output_aliases=(),
                sim_require_finite=True,
                sim_require_nnan=True,
                nc=nc,
            )
            return tuple(outs)

        devices = jax.devices()[:n_cores]
        mesh = Mesh(np.asarray(devices), ("core",))
        in_specs = (PartitionSpec("core"),) * (n_params + n_outs)
        out_specs = (PartitionSpec("core"),) * len(out_names)
        self.fn = jax.jit(
            shard_map(_body, mesh=mesh, in_specs=in_specs, out_specs=out_specs,
                      check_rep=False),
            keep_unused=True)

    def run(self, in_maps):
        concat_in = [np.concatenate([np.asarray(m[n]) for m in in_maps], axis=0)
                     for n in self.in_names]
        concat_zero = [np.zeros((self.n_cores * z.shape[0], *z.shape[1:]), z.dtype)
                       for z in self.zero_outs]
        outs = self.fn(*concat_in, *concat_zero)
        res = []
        for c in range(self.n_cores):
            d = {}
            for i, n in enumerate(self.out_names):
                a = np.asarray(outs[i])
                d[n] = a.reshape(self.n_cores, *self.out_avals[i].shape)[c]
            res.append(d)
        return res


_CACHE = {}


def _get_runner():
    if "r" not in _CACHE:
        nc = _build_nc()
        _CACHE["r"] = _Runner(nc, N_CORES)
    return _CACHE["r"]


def kernel(**inputs):
    runner = _get_runner()
    in_maps = [_host_prep(inputs, c) for c in range(N_CORES)]
    res = runner.run(in_maps)
    outs = np.zeros((B, V, T), np.float32)
    attns = np.zeros((B, T, L), np.float32)
    for c in range(N_CORES):
        o, a = _host_post(res[c])
        outs[c * NB:(c + 1) * NB] = o
        attns[c * NB:(c + 1) * NB] = a
    return outs, attns


# revision 2
# speedup vs baseline: 3503.6675x; 3503.6675x over previous
"""nn_AttnDecoder TRN2 kernel: 8-core data-parallel (batch-sharded) Bass kernel.

Self-contained: builds/compiles the bass program on first call (cached),
shards inputs across 8 NeuronCores, runs via PJRT, reassembles full outputs.
"""
import numpy as np
import ml_dtypes
from contextlib import ExitStack

import jax
from jax.sharding import Mesh, PartitionSpec
from jax.experimental.shard_map import shard_map

import concourse.bass as bass
import concourse.tile as tile
from concourse import bacc, mybir
from concourse.bass2jax import (_bass_exec_p, install_neuronx_cc_hook,
                                partition_id_tensor)

F32 = mybir.dt.float32
F16 = mybir.dt.float16
BF16 = mybir.dt.bfloat16
AF = mybir.ActivationFunctionType
OP = mybir.AluOpType
BF = ml_dtypes.bfloat16

B, H, L, T, V, LAYERS = 512, 256, 256, 64, 128, 2
NB = 64
N_CORES = 8


def _build_nc(Tn=T, nb=NB, n_devices=N_CORES):
    nc = bacc.Bacc("TRN2", target_bir_lowering=False, debug=False,
                   num_devices=n_devices)
    TB = Tn * nb

    d_x = nc.dram_tensor("x_r", [nb, 2, 128, L], BF16, kind="ExternalInput").ap()
    d_c1t = nc.dram_tensor("c1t", [128, 2 * H], BF16, kind="ExternalInput").ap()
    d_a2t = nc.dram_tensor("a2t", [128, 4 * 128], BF16, kind="ExternalInput").ap()
    d_ea1 = nc.dram_tensor("ea1", [128, 2 * TB], BF16, kind="ExternalInput").ap()
    d_c2e = nc.dram_tensor("c2e", [128, 2 * TB], BF16, kind="ExternalInput").ap()
    d_wrz = nc.dram_tensor("wrz", [128, LAYERS * 16 * 128], BF16, kind="ExternalInput").ap()
    d_win = nc.dram_tensor("win", [128, LAYERS * 4 * 128], BF16, kind="ExternalInput").ap()
    d_whn = nc.dram_tensor("whn", [128, LAYERS * 4 * 128], BF16, kind="ExternalInput").ap()
    d_owt = nc.dram_tensor("owt", [128, 2 * V], BF16, kind="ExternalInput").ap()
    d_outb = nc.dram_tensor("outb", [128, 1], F32, kind="ExternalInput").ap()
    d_onescol = nc.dram_tensor("onescol", [128, 1], BF16, kind="ExternalInput").ap()
    d_onesrow = nc.dram_tensor("onesrow", [1, 128], BF16, kind="ExternalInput").ap()

    d_logp = nc.dram_tensor("out_logp", [V, TB], F32, kind="ExternalOutput").ap()
    d_attn = nc.dram_tensor("out_attn", [2 * 128, TB], F32, kind="ExternalOutput").ap()

    with tile.TileContext(nc) as tc, ExitStack() as ctx:
        st = ctx.enter_context(tc.tile_pool(name="statics", bufs=1))
        work = ctx.enter_context(tc.tile_pool(name="work", bufs=2))
        xs = ctx.enter_context(tc.tile_pool(name="xs", bufs=3))
        psst = ctx.enter_context(tc.tile_pool(name="psst", bufs=1, space="PSUM"))

        PB = []
        for i in range(8):
            pb = psst.tile([128, 512], F32, tag=f"pbank{i}", name=f"pbank{i}")
            PB.append(pb)
        (PB_SC, PB_CP, PB_RZ0, PB_RZ1, PB_S1, PB_HN, PB_BZ, PB_LG) = PB

        c1t = st.tile([128, 2 * H], BF16)
        a2t = st.tile([128, 4 * 128], BF16)
        ea1 = st.tile([128, 2 * TB], BF16)
        c2e = st.tile([128, 2 * TB], BF16)
        wrz = st.tile([128, LAYERS * 16 * 128], BF16)
        win = st.tile([128, LAYERS * 4 * 128], BF16)
        whn = st.tile([128, LAYERS * 4 * 128], BF16)
        owt = st.tile([128, 2 * V], BF16)
        outb = st.tile([128, 1], F32)
        onescol = st.tile([128, 1], BF16)
        onesrow = st.tile([1, 128], BF16)
        onesrow16 = st.tile([1, 128], F16)
        for ap_d, ap_s in ((d_c1t, c1t), (d_a2t, a2t), (d_ea1, ea1), (d_c2e, c2e),
                           (d_wrz, wrz), (d_win, win), (d_whn, whn), (d_owt, owt),
                           (d_outb, outb), (d_onescol, onescol), (d_onesrow, onesrow)):
            nc.sync.dma_start(ap_s[:], ap_d[:])
        nc.vector.tensor_copy(onesrow16[:], onesrow[:])

        mbt = st.tile([128, nb * 2 * H], BF16)
        h1h = st.tile([128, (Tn + 1) * 2 * nb], BF16)
        exph = st.tile([128, 2 * TB], BF16)
        rzh = st.tile([1, TB], BF16)
        h0s = st.tile([128, 2 * 2 * nb], BF16)
        h0m = st.tile([128, 2 * 2 * nb], F32)
        h1m = st.tile([128, 2 * 2 * nb], F32)
        nc.vector.memset(h0s[:], 0.0)
        nc.vector.memset(h0m[:], 0.0)
        nc.vector.memset(h1m[:], 0.0)
        nc.vector.memset(h1h[:, 0:2 * nb], 0.0)

        with nc.named_scope("mbt"):
            for b in range(nb):
                x16 = xs.tile([128, 2 * L], BF16, tag="x16", name="x16")
                for hc in range(2):
                    nc.sync.dma_start(x16[:, hc * L:(hc + 1) * L], d_x[b, hc])
                for lc2 in range(2):
                    mps = PB_LG[:, (b % 2) * 256:(b % 2) * 256 + H]
                    for hc in range(2):
                        nc.tensor.matmul(
                            mps,
                            x16[:, hc * L + lc2 * 128: hc * L + lc2 * 128 + 128],
                            c1t[:, hc * H:(hc + 1) * H],
                            start=(hc == 0), stop=(hc == 1))
                    dst = mbt[:, (b * 2 + lc2) * H:(b * 2 + lc2 + 1) * H]
                    if lc2 == 0:
                        nc.vector.tensor_copy(dst, mps)
                    else:
                        nc.scalar.copy(dst, mps)

        for t in range(Tn):
            pp, qq = t % 2, 1 - (t % 2)
            h0sh_prev = h0s[:, pp * 2 * nb:(pp + 1) * 2 * nb]
            h1sh_prev = h1h[:, t * 2 * nb:(t + 1) * 2 * nb]
            with nc.named_scope(f"s{t}"):
                sc = PB_SC[:, pp * 128: pp * 128 + 128]
                for lc in range(2):
                    for hc in range(2):
                        nc.tensor.matmul(
                            sc[:, lc * nb:(lc + 1) * nb],
                            a2t[:, (hc * 2 + lc) * 128:(hc * 2 + lc + 1) * 128],
                            h0sh_prev[:, hc * nb:(hc + 1) * nb],
                            start=(hc == 0), stop=(hc == 1))
                es = work.tile([128, 2 * nb], BF16, tag="es", name="es")
                nc.scalar.activation(es[:], sc, AF.Exp)
                eslot = exph[:, t * 2 * nb:(t + 1) * 2 * nb]
                nc.vector.tensor_tensor(eslot, es[:],
                                        ea1[:, t * 2 * nb:(t + 1) * 2 * nb], OP.mult)
                zrow = PB_BZ[0:1, pp * 256: pp * 256 + nb]
                for lc in range(2):
                    nc.tensor.matmul(zrow, onescol[:, 0:1],
                                     exph[:, (t * 2 + lc) * nb:(t * 2 + lc + 1) * nb],
                                     start=(lc == 0), stop=(lc == 1))
                rz16 = rzh[0:1, t * nb:(t + 1) * nb]
                with nc.allow_low_precision(reason="recipZ bf16 is plenty"):
                    nc.vector.reciprocal(rz16, zrow)
                cp = PB_CP[:, pp * 128: pp * 128 + 128]
                for b in range(nb):
                    for hc2 in range(2):
                        for lc in range(2):
                            nc.tensor.matmul(
                                cp[:, hc2 * nb + b: hc2 * nb + b + 1],
                                mbt[:, (b * 2 + lc) * H + hc2 * 128:
                                       (b * 2 + lc) * H + hc2 * 128 + 128],
                                exph[:, (t * 2 + lc) * nb + b:
                                        (t * 2 + lc) * nb + b + 1],
                                start=(lc == 0), stop=(lc == 1))
                bcp = PB_BZ[:, pp * 256 + 64: pp * 256 + 128]
                nc.tensor.matmul(bcp, onesrow[:, :], rz16, start=True, stop=True)
                bcs = work.tile([128, nb], F32, tag="bcs", name="bcs")
                nc.scalar.copy(bcs[:], bcp)
                t1 = work.tile([128, 2 * nb], BF16, tag="t1", name="t1")
                a_big, a_small = bass.broadcast_tensor_aps(
                    cp.rearrange("p (c b) -> p c b", c=2),
                    bcs[:].rearrange("p (c b) -> p c b", c=1))
                nc.vector.tensor_tensor(
                    t1[:].rearrange("p (c b) -> p c b", c=2), a_big, a_small, OP.mult)
                t2 = work.tile([128, 2 * nb], BF16, tag="t2", name="t2")
                nc.vector.tensor_tensor(t2[:], t1[:],
                                        c2e[:, t * 2 * nb:(t + 1) * 2 * nb], OP.add)
                g16 = work.tile([128, 2 * nb], BF16, tag="g16", name="g16")
                nc.vector.tensor_scalar_max(g16[:], t2[:], 0.0)

                xin = g16[:]
                for layer in range(LAYERS):
                    hm = h0m if layer == 0 else h1m
                    hsh_prev = h0sh_prev if layer == 0 else h1sh_prev
                    hm_prev = hm[:, pp * 2 * nb:(pp + 1) * 2 * nb]
                    rzp = (PB_RZ0 if layer == 0 else PB_RZ1)[:, pp * 256: pp * 256 + 256]
                    for mc in range(4):
                        for kc in range(4):
                            rhs = (xin[:, (kc % 2) * nb:(kc % 2 + 1) * nb] if kc < 2
                                   else hsh_prev[:, (kc - 2) * nb:(kc - 1) * nb])
                            nc.tensor.matmul(
                                rzp[:, mc * nb:(mc + 1) * nb],
                                wrz[:, (layer * 16 + kc * 4 + mc) * 128:
                                       (layer * 16 + kc * 4 + mc + 1) * 128],
                                rhs, start=(kc == 0), stop=(kc == 3))
                    s1p = PB_S1[:, (pp * 2 + layer) * 128:(pp * 2 + layer) * 128 + 128]
                    hnp = PB_HN[:, (pp * 2 + layer) * 128:(pp * 2 + layer) * 128 + 128]
                    for mc in range(2):
                        for kc in range(2):
                            nc.tensor.matmul(
                                s1p[:, mc * nb:(mc + 1) * nb],
                                win[:, (layer * 4 + kc * 2 + mc) * 128:
                                       (layer * 4 + kc * 2 + mc + 1) * 128],
                                xin[:, kc * nb:(kc + 1) * nb],
                                start=(kc == 0), stop=(kc == 1))
                            nc.tensor.matmul(
                                hnp[:, mc * nb:(mc + 1) * nb],
                                whn[:, (layer * 4 + kc * 2 + mc) * 128:
                                       (layer * 4 + kc * 2 + mc + 1) * 128],
                                hsh_prev[:, kc * nb:(kc + 1) * nb],
                                start=(kc == 0), stop=(kc == 1))
                    trz = work.tile([128, 4 * nb], BF16, tag=f"trz{layer}", name="trz")
                    nc.scalar.activation(trz[:], rzp, AF.Tanh, scale=0.5)
                    u2 = work.tile([128, 2 * nb], F32, tag=f"u2{layer}", name="u2")
                    nc.vector.scalar_tensor_tensor(u2[:], trz[:, 0:2 * nb], 1.0, hnp,
                                                   OP.add, OP.mult)
                    vv = work.tile([128, 2 * nb], F32, tag=f"vv{layer}", name="vv")
                    nc.vector.tensor_tensor(vv[:], s1p, u2[:], OP.add)
                    nn_t = work.tile([128, 2 * nb], BF16, tag=f"nn{layer}", name="nn")
                    nc.scalar.activation(nn_t[:], vv[:], AF.Tanh)
                    q_t = work.tile([128, 2 * nb], BF16, tag=f"q{layer}", name="q")
                    nc.vector.tensor_tensor(q_t[:], hsh_prev, nn_t[:], OP.subtract)
                    p_t = work.tile([128, 2 * nb], F32, tag=f"p{layer}", name="p")
                    nc.vector.tensor_tensor(p_t[:], hm_prev, nn_t[:], OP.add)
                    r2 = work.tile([128, 2 * nb], BF16, tag=f"r2{layer}", name="r2")
                    nc.vector.tensor_tensor(r2[:], q_t[:], trz[:, 2 * nb:4 * nb], OP.mult)
                    tt = work.tile([128, 2 * nb], F32, tag=f"tt{layer}", name="tt")
                    nc.vector.tensor_tensor(tt[:], p_t[:], r2[:], OP.add)
                    hm_new = hm[:, qq * 2 * nb:(qq + 1) * 2 * nb]
                    nc.vector.tensor_scalar_mul(hm_new, tt[:], 0.5)
                    if layer == 0:
                        sh_new = h0s[:, qq * 2 * nb:(qq + 1) * 2 * nb]
                    else:
                        sh_new = h1h[:, (t + 1) * 2 * nb:(t + 2) * 2 * nb]
                    nc.vector.tensor_scalar_mul(sh_new, tt[:], 0.5)
                    xin = sh_new

        NCH = min(512, TB)
        h1h4 = h1h[:].rearrange("p (t c b) -> p t c b", c=2, b=nb)
        with nc.named_scope("logits"):
            for ch in range(TB // NCH):
                tpc = NCH // nb
                lp = (PB_LG if ch % 2 == 0 else PB_CP)[:, 0:NCH]
                for kc in range(2):
                    rhs = h1h4[:, 1 + ch * tpc: 1 + (ch + 1) * tpc, kc, :]
                    nc.tensor.matmul(lp, owt[:, kc * V:(kc + 1) * V], rhs,
                                     start=(kc == 0), stop=(kc == 1))
                ex16 = work.tile([128, NCH], BF16, tag="ex16", name="ex16")
                nc.scalar.activation(ex16[:], lp, AF.Exp, bias=outb[:, 0:1])
                zr = (PB_SC if ch % 2 == 0 else PB_RZ0)[0:1, 0:NCH]
                nc.tensor.matmul(zr, onescol[:, 0:1], ex16[:], start=True, stop=True)
                lnz = work.tile([1, NCH], F16, tag="lnz", name="lnz")
                nc.scalar.activation(lnz[:], zr, AF.Ln)
                bcl = (PB_S1 if ch % 2 == 0 else PB_HN)[:, 0:NCH]
                nc.tensor.matmul(bcl, onesrow16[:, :], lnz[:], start=True, stop=True)
                bcl_s = work.tile([128, NCH], F16, tag="bcl_s", name="bcl_s")
                nc.scalar.copy(bcl_s[:], bcl)
                lout = work.tile([128, NCH], F32, tag="lout", name="lout")
                nc.vector.scalar_tensor_tensor(lout[:], lp, outb[:, 0:1], bcl_s[:],
                                               OP.add, OP.subtract)
                nc.sync.dma_start(d_logp[:, ch * NCH:(ch + 1) * NCH], lout[:])

        exph4 = exph[:].rearrange("p (t c b) -> p t c b", c=2, b=nb)
        with nc.named_scope("attn"):
            for ch in range(TB // NCH):
                tpc = NCH // nb
                bca = (PB_RZ1 if ch % 2 == 0 else PB_BZ)[:, 0:NCH]
                nc.tensor.matmul(bca, onesrow[:, :],
                                 rzh[0:1, ch * NCH:(ch + 1) * NCH],
                                 start=True, stop=True)
                for lc in range(2):
                    an = work.tile([128, NCH], F32, tag="an", name="an")
                    nc.vector.tensor_tensor(
                        an[:], exph4[:, ch * tpc:(ch + 1) * tpc, lc, :], bca, OP.mult)
                    nc.sync.dma_start(
                        d_attn[lc * 128:(lc + 1) * 128, ch * NCH:(ch + 1) * NCH], an[:])

    nc.compile()
    return nc


def _host_prep(inputs, core, Tn=T, nb=NB):
    x = np.asarray(inputs["x"])[:, :, 0, :]
    y = np.asarray(inputs["y"]).astype(np.int64)
    emb = np.asarray(inputs["emb"], np.float32)
    attn_w = np.asarray(inputs["attn_w"], np.float32)
    attn_b = np.asarray(inputs["attn_b"], np.float32)
    comb_w = np.asarray(inputs["comb_w"], np.float32)
    comb_b = np.asarray(inputs["comb_b"], np.float32)
    gwih = np.asarray(inputs["gru_w_ih"], np.float32)
    gwhh = np.asarray(inputs["gru_w_hh"], np.float32)
    gbih = np.asarray(inputs["gru_b_ih"], np.float32)
    gbhh = np.asarray(inputs["gru_b_hh"], np.float32)
    out_w = np.asarray(inputs["out_w"], np.float32)
    out_b = np.asarray(inputs["out_b"], np.float32)
    assert np.all(gbih == 0) and np.all(gbhh == 0), "nonzero GRU biases unsupported"

    sl = slice(core * nb, (core + 1) * nb)
    tokens = np.concatenate([np.zeros((B, 1), y.dtype), y[:, :-1]], axis=1)[sl]
    xb = x[sl]
    A1, A2 = attn_w[:, :H], attn_w[:, H:]
    C1, C2 = comb_w[:, :H], comb_w[:, H:]

    ea1_tab = np.exp(A1 @ emb.T + attn_b[:, None]).astype(np.float32)
    c2e_tab = (C2 @ emb.T + comb_b[:, None]).astype(np.float32)

    tb = Tn * nb
    ea1_g = ea1_tab[:, tokens.T[:Tn]]
    ea1_r = ea1_g.reshape(2, 128, Tn, nb).transpose(1, 2, 0, 3).reshape(128, 2 * tb)
    c2e_g = c2e_tab[:, tokens.T[:Tn]]
    c2e_r = c2e_g.reshape(2, 128, Tn, nb).transpose(1, 2, 0, 3).reshape(128, 2 * tb)

    def lhs_blocks(Wt, n_kc, n_mc):
        K, M = Wt.shape
        assert K == n_kc * 128 and M == n_mc * 128
        a = Wt.reshape(n_kc, 128, n_mc, 128).transpose(1, 0, 2, 3)
        return a.reshape(128, n_kc * n_mc * 128)

    c1t_rhs = C1.T.reshape(2, 128, 256).transpose(1, 0, 2).reshape(128, 2 * H)
    a2t_r = lhs_blocks(A2.T, 2, 2)

    wrz_list, win_list, whn_list = [], [], []
    for lyr in range(LAYERS):
        Wi, Wh = gwih[lyr], gwhh[lyr]
        lhs_rz = np.vstack([Wi[0:2 * H, :].T, Wh[0:2 * H, :].T])
        wrz_list.append(lhs_blocks(lhs_rz, 4, 4))
        win_list.append(lhs_blocks(Wi[2 * H:3 * H, :].T, 2, 2))
        whn_list.append(lhs_blocks(0.5 * Wh[2 * H:3 * H, :].T, 2, 2))
    wrz_r = np.concatenate(wrz_list, axis=1)
    win_r = np.concatenate(win_list, axis=1)
    whn_r = np.concatenate(whn_list, axis=1)
    owt_r = lhs_blocks(out_w.T, 2, 1)

    return {
        "x_r": np.ascontiguousarray(xb.reshape(nb, 2, 128, L)).astype(BF),
        "c1t": c1t_rhs.astype(BF),
        "a2t": a2t_r.astype(BF),
        "ea1": ea1_r.astype(BF),
        "c2e": c2e_r.astype(BF),
        "wrz": wrz_r.astype(BF),
        "win": win_r.astype(BF),
        "whn": whn_r.astype(BF),
        "owt": owt_r.astype(BF),
        "outb": out_b.reshape(128, 1).astype(np.float32),
        "onescol": np.ones((128, 1), BF),
        "onesrow": np.ones((1, 128), BF),
    }


def _host_post(res, Tn=T, nb=NB):
    lp = res["out_logp"].reshape(V, Tn, nb)
    outs = np.ascontiguousarray(lp.transpose(2, 0, 1))
    at = res["out_attn"].reshape(2, 128, Tn, nb)
    attns = np.ascontiguousarray(at.transpose(3, 2, 0, 1).reshape(nb, Tn, L))
    return outs, attns


class _Runner:
    def __init__(self, nc, n_cores):
        install_neuronx_cc_hook()
        self.nc = nc
        self.n_cores = n_cores
        in_names, out_names, out_avals, zero_outs = [], [], [], []
        partition_name = nc.partition_id_tensor.name if nc.partition_id_tensor else None
        for alloc in nc.m.functions[0].allocations:
            if not isinstance(alloc, mybir.MemoryLocationSet):
                continue
            name = alloc.memorylocations[0].name
            if alloc.kind == "ExternalInput":
                if name != partition_name:
                    in_names.append(name)
            elif alloc.kind == "ExternalOutput":
                shape = tuple(alloc.tensor_shape)
                dtype = mybir.dt.np(alloc.dtype)
                out_names.append(name)
                out_avals.append(jax.core.ShapedArray(shape, dtype))
                zero_outs.append(np.zeros(shape, dtype))
        self.in_names, self.out_names = in_names, out_names
        self.out_avals, self.zero_outs = out_avals, zero_outs
        n_params, n_outs = len(in_names), len(out_avals)
        all_in_names = list(in_names) + list(out_names)
        if partition_name is not None:
            all_in_names.append(partition_name)

        def _body(*args):
            operands = list(args)
            if partition_name is not None:
                operands.append(partition_id_tensor())
            outs = _bass_exec_p.bind(
                *operands,
                out_avals=tuple(out_avals),
                in_names=tuple(all_in_names),
                out_names=tuple(out_names),
                lowering_input_output_aliases=(),
                sim_require_finite=True,
                sim_require_nnan=True,
                nc=nc,
            )
            return tuple(outs)

        devices = jax.devices()[:n_cores]
        mesh = Mesh(np.asarray(devices), ("core",))
        in_specs = (PartitionSpec("core"),) * (n_params + n_outs)
        out_specs = (PartitionSpec("core"),) * len(out_names)
        self.fn = jax.jit(
            shard_map(_body, mesh=mesh, in_specs=in_specs, out_specs=out_specs,
                      check_rep=False),
            keep_unused=True)

    def run(self, in_maps):
        concat_in = [np.concatenate([np.asarray(m[n]) for m in in_maps], axis=0)
                     for n in self.in_names]
        concat_zero = [np.zeros((self.n_cores * z.shape[0], *z.shape[1:]), z.dtype)
                       for z in self.zero_outs]
        outs = self.fn(*concat_in, *concat_zero)
        res = []
        for c in range(self.n_cores):
            d = {}
            for i, n in enumerate(self.out_names):
                a = np.asarray(outs[i])
                d[n] = a.reshape(self.n_cores, *self.out_avals[i].shape)[c]
            res.append(d)
        return res


_CACHE = {}


def _get_runner():
    if "r" not in _CACHE:
        nc = _build_nc()
        _CACHE["r"] = _Runner(nc, N_CORES)
    return _CACHE["r"]


def kernel(**inputs):
    runner = _get_runner()
    in_maps = [_host_prep(inputs, c) for c in range(N_CORES)]
    res = runner.run(in_maps)
    outs = np.zeros((B, V, T), np.float32)
    attns = np.zeros((B, T, L), np.float32)
    for c in range(N_CORES):
        o, a = _host_post(res[c])
        outs[c * NB:(c + 1) * NB] = o
        attns[c * NB:(c + 1) * NB] = a
    return outs, attns


# revision 4
# speedup vs baseline: 3699.0465x; 1.0558x over previous
"""nn_AttnDecoder TRN2 kernel: 8-core data-parallel (batch-sharded) Bass kernel.

Self-contained: builds/compiles the bass program on first call (cached),
shards inputs across 8 NeuronCores, runs via PJRT, reassembles full outputs.
"""
import numpy as np
import ml_dtypes
from contextlib import ExitStack

import jax
from jax.sharding import Mesh, PartitionSpec
from jax.experimental.shard_map import shard_map

import concourse.bass as bass
import concourse.tile as tile
from concourse import bacc, mybir
from concourse.bass2jax import (_bass_exec_p, install_neuronx_cc_hook,
                                partition_id_tensor)

F32 = mybir.dt.float32
F16 = mybir.dt.float16
BF16 = mybir.dt.bfloat16
AF = mybir.ActivationFunctionType
OP = mybir.AluOpType
BF = ml_dtypes.bfloat16

B, H, L, T, V, LAYERS = 512, 256, 256, 64, 128, 2
NB = 64
N_CORES = 8


def _build_nc(Tn=T, nb=NB, n_devices=N_CORES):
    nc = bacc.Bacc("TRN2", target_bir_lowering=False, debug=False,
                   num_devices=n_devices)
    TB = Tn * nb

    d_x = nc.dram_tensor("x_r", [nb, 2, 128, L], BF16, kind="ExternalInput").ap()
    d_c1t = nc.dram_tensor("c1t", [128, 2 * H], BF16, kind="ExternalInput").ap()
    d_a2t = nc.dram_tensor("a2t", [128, 4 * 128], BF16, kind="ExternalInput").ap()
    d_ea1 = nc.dram_tensor("ea1", [128, 2 * TB], BF16, kind="ExternalInput").ap()
    d_c2e = nc.dram_tensor("c2e", [128, 2 * TB], BF16, kind="ExternalInput").ap()
    d_wrz = nc.dram_tensor("wrz", [128, LAYERS * 16 * 128], BF16, kind="ExternalInput").ap()
    d_win = nc.dram_tensor("win", [128, LAYERS * 4 * 128], BF16, kind="ExternalInput").ap()
    d_whn = nc.dram_tensor("whn", [128, LAYERS * 4 * 128], BF16, kind="ExternalInput").ap()
    d_owt = nc.dram_tensor("owt", [128, 2 * V], BF16, kind="ExternalInput").ap()
    d_outb = nc.dram_tensor("outb", [128, 1], F32, kind="ExternalInput").ap()
    d_onescol = nc.dram_tensor("onescol", [128, 1], BF16, kind="ExternalInput").ap()
    d_onesrow = nc.dram_tensor("onesrow", [1, 128], BF16, kind="ExternalInput").ap()

    d_logp = nc.dram_tensor("out_logp", [V, TB], F32, kind="ExternalOutput").ap()
    d_attn = nc.dram_tensor("out_attn", [2 * 128, TB], F32, kind="ExternalOutput").ap()

    with tile.TileContext(nc) as tc, ExitStack() as ctx:
        st = ctx.enter_context(tc.tile_pool(name="statics", bufs=1))
        work = ctx.enter_context(tc.tile_pool(name="work", bufs=2))
        xs = ctx.enter_context(tc.tile_pool(name="xs", bufs=3))
        psst = ctx.enter_context(tc.tile_pool(name="psst", bufs=1, space="PSUM"))

        PB = []
        for i in range(8):
            pb = psst.tile([128, 512], F32, tag=f"pbank{i}", name=f"pbank{i}")
            PB.append(pb)
        (PB_SC, PB_CP, PB_RZ0, PB_RZ1, PB_S1, PB_HN, PB_BZ, PB_LG) = PB

        c1t = st.tile([128, 2 * H], BF16)
        a2t = st.tile([128, 4 * 128], BF16)
        ea1 = st.tile([128, 2 * TB], BF16)
        c2e = st.tile([128, 2 * TB], BF16)
        wrz = st.tile([128, LAYERS * 16 * 128], BF16)
        win = st.tile([128, LAYERS * 4 * 128], BF16)
        whn = st.tile([128, LAYERS * 4 * 128], BF16)
        owt = st.tile([128, 2 * V], BF16)
        outb = st.tile([128, 1], F32)
        onescol = st.tile([128, 1], BF16)
        onesrow = st.tile([1, 128], BF16)
        onesrow16 = st.tile([1, 128], F16)
        for ap_d, ap_s in ((d_c1t, c1t), (d_a2t, a2t), (d_ea1, ea1), (d_c2e, c2e),
                           (d_wrz, wrz), (d_win, win), (d_whn, whn), (d_owt, owt),
                           (d_outb, outb), (d_onescol, onescol), (d_onesrow, onesrow)):
            nc.sync.dma_start(ap_s[:], ap_d[:])
        nc.vector.tensor_copy(onesrow16[:], onesrow[:])

        mbt = st.tile([128, nb * 2 * H], BF16)
        h1h = st.tile([128, (Tn + 1) * 2 * nb], BF16)
        exph = st.tile([128, 2 * TB], BF16)
        rzh = st.tile([1, TB], BF16)
        h0s = st.tile([128, 2 * 2 * nb], BF16)
        h0m = st.tile([128, 2 * 2 * nb], F32)
        h1m = st.tile([128, 2 * 2 * nb], F32)
        nc.vector.memset(h0s[:], 0.0)
        nc.vector.memset(h0m[:], 0.0)
        nc.vector.memset(h1m[:], 0.0)
        nc.vector.memset(h1h[:, 0:2 * nb], 0.0)

        with nc.named_scope("mbt"):
            for b in range(nb):
                x16 = xs.tile([128, 2 * L], BF16, tag="x16", name="x16")
                for hc in range(2):
                    nc.sync.dma_start(x16[:, hc * L:(hc + 1) * L], d_x[b, hc])
                for lc2 in range(2):
                    mps = PB_LG[:, (b % 2) * 256:(b % 2) * 256 + H]
                    for hc in range(2):
                        nc.tensor.matmul(
                            mps,
                            x16[:, hc * L + lc2 * 128: hc * L + lc2 * 128 + 128],
                            c1t[:, hc * H:(hc + 1) * H],
                            start=(hc == 0), stop=(hc == 1))
                    dst = mbt[:, (b * 2 + lc2) * H:(b * 2 + lc2 + 1) * H]
                    if lc2 == 0:
                        nc.vector.tensor_copy(dst, mps)
                    else:
                        nc.scalar.copy(dst, mps)

        for t in range(Tn):
            pp, qq = t % 2, 1 - (t % 2)
            h0sh_prev = h0s[:, pp * 2 * nb:(pp + 1) * 2 * nb]
            h1sh_prev = h1h[:, t * 2 * nb:(t + 1) * 2 * nb]
            with nc.named_scope(f"s{t}"):
                sc = PB_SC[:, pp * 128: pp * 128 + 128]
                for lc in range(2):
                    for hc in range(2):
                        nc.tensor.matmul(
                            sc[:, lc * nb:(lc + 1) * nb],
                            a2t[:, (hc * 2 + lc) * 128:(hc * 2 + lc + 1) * 128],
                            h0sh_prev[:, hc * nb:(hc + 1) * nb],
                            start=(hc == 0), stop=(hc == 1))
                es = work.tile([128, 2 * nb], BF16, tag="es", name="es")
                nc.scalar.activation(es[:], sc, AF.Exp)
                eslot = exph[:, t * 2 * nb:(t + 1) * 2 * nb]
                nc.vector.tensor_tensor(eslot, es[:],
                                        ea1[:, t * 2 * nb:(t + 1) * 2 * nb], OP.mult)
                zrow = PB_BZ[0:1, pp * 256: pp * 256 + nb]
                for lc in range(2):
                    nc.tensor.matmul(zrow, onescol[:, 0:1],
                                     exph[:, (t * 2 + lc) * nb:(t * 2 + lc + 1) * nb],
                                     start=(lc == 0), stop=(lc == 1))
                rz16 = rzh[0:1, t * nb:(t + 1) * nb]
                with nc.allow_low_precision(reason="recipZ bf16 is plenty"):
                    nc.vector.reciprocal(rz16, zrow)
                cp = PB_CP[:, pp * 128: pp * 128 + 128]
                for b in range(nb):
                    for hc2 in range(2):
                        for lc in range(2):
                            nc.tensor.matmul(
                                cp[:, hc2 * nb + b: hc2 * nb + b + 1],
                                mbt[:, (b * 2 + lc) * H + hc2 * 128:
                                       (b * 2 + lc) * H + hc2 * 128 + 128],
                                exph[:, (t * 2 + lc) * nb + b:
                                        (t * 2 + lc) * nb + b + 1],
                                start=(lc == 0), stop=(lc == 1))
                bcp = PB_BZ[:, pp * 256 + 64: pp * 256 + 128]
                nc.tensor.matmul(bcp, onesrow[:, :], rz16, start=True, stop=True)
                bcs = work.tile([128, nb], F32, tag="bcs", name="bcs")
                nc.scalar.copy(bcs[:], bcp)
                t1 = work.tile([128, 2 * nb], BF16, tag="t1", name="t1")
                a_big, a_small = bass.broadcast_tensor_aps(
                    cp.rearrange("p (c b) -> p c b", c=2),
                    bcs[:].rearrange("p (c b) -> p c b", c=1))
                nc.vector.tensor_tensor(
                    t1[:].rearrange("p (c b) -> p c b", c=2), a_big, a_small, OP.mult)
                t2 = work.tile([128, 2 * nb], BF16, tag="t2", name="t2")
                nc.vector.tensor_tensor(t2[:], t1[:],
                                        c2e[:, t * 2 * nb:(t + 1) * 2 * nb], OP.add)
                g16 = work.tile([128, 2 * nb], BF16, tag="g16", name="g16")
                nc.vector.tensor_scalar_max(g16[:], t2[:], 0.0)

                xin = g16[:]
                for layer in range(LAYERS):
                    hm = h0m if layer == 0 else h1m
                    hsh_prev = h0sh_prev if layer == 0 else h1sh_prev
                    hm_prev = hm[:, pp * 2 * nb:(pp + 1) * 2 * nb]
                    rzp = (PB_RZ0 if layer == 0 else PB_RZ1)[:, pp * 256: pp * 256 + 256]
                    for mc in range(4):
                        for kc in range(4):
                            rhs = (xin[:, (kc % 2) * nb:(kc % 2 + 1) * nb] if kc < 2
                                   else hsh_prev[:, (kc - 2) * nb:(kc - 1) * nb])
                            nc.tensor.matmul(
                                rzp[:, mc * nb:(mc + 1) * nb],
                                wrz[:, (layer * 16 + kc * 4 + mc) * 128:
                                       (layer * 16 + kc * 4 + mc + 1) * 128],
                                rhs, start=(kc == 0), stop=(kc == 3))
                    s1p = PB_S1[:, (pp * 2 + layer) * 128:(pp * 2 + layer) * 128 + 128]
                    hnp = PB_HN[:, (pp * 2 + layer) * 128:(pp * 2 + layer) * 128 + 128]
                    for mc in range(2):
                        for kc in range(2):
                            nc.tensor.matmul(
                                s1p[:, mc * nb:(mc + 1) * nb],
                                win[:, (layer * 4 + kc * 2 + mc) * 128:
                                       (layer * 4 + kc * 2 + mc + 1) * 128],
                                xin[:, kc * nb:(kc + 1) * nb],
                                start=(kc == 0), stop=(kc == 1))
                            nc.tensor.matmul(
                                hnp[:, mc * nb:(mc + 1) * nb],
                                whn[:, (layer * 4 + kc * 2 + mc) * 128:
                                       (layer * 4 + kc * 2 + mc + 1) * 128],
                                hsh_prev[:, kc * nb:(kc + 1) * nb],
                                start=(kc == 0), stop=(kc == 1))
                    trz = work.tile([128, 4 * nb], BF16, tag=f"trz{layer}", name="trz")
                    nc.scalar.activation(trz[:], rzp, AF.Tanh, scale=0.5)
                    u2 = work.tile([128, 2 * nb], F32, tag=f"u2{layer}", name="u2")
                    nc.vector.scalar_tensor_tensor(u2[:], trz[:, 0:2 * nb], 1.0, hnp,
                                                   OP.add, OP.mult)
                    vv = work.tile([128, 2 * nb], F32, tag=f"vv{layer}", name="vv")
                    nc.vector.tensor_tensor(vv[:], s1p, u2[:], OP.add)
                    nn_t = work.tile([128, 2 * nb], BF16, tag=f"nn{layer}", name="nn")
                    nc.scalar.activation(nn_t[:], vv[:], AF.Tanh)
                    q_t = work.tile([128, 2 * nb], BF16, tag=f"q{layer}", name="q")
                    nc.vector.tensor_tensor(q_t[:], hsh_prev, nn_t[:], OP.subtract)
                    p_t = work.tile([128, 2 * nb], F32, tag=f"p{layer}", name="p")
                    nc.vector.tensor_tensor(p_t[:], hm_prev, nn_t[:], OP.add)
                    r2 = work.tile([128, 2 * nb], BF16, tag=f"r2{layer}", name="r2")
                    nc.vector.tensor_tensor(r2[:], q_t[:], trz[:, 2 * nb:4 * nb], OP.mult)
                    tt = work.tile([128, 2 * nb], F32, tag=f"tt{layer}", name="tt")
                    nc.vector.tensor_tensor(tt[:], p_t[:], r2[:], OP.add)
                    hm_new = hm[:, qq * 2 * nb:(qq + 1) * 2 * nb]
                    nc.vector.tensor_scalar_mul(hm_new, tt[:], 0.5)
                    if layer == 0:
                        sh_new = h0s[:, qq * 2 * nb:(qq + 1) * 2 * nb]
                    else:
                        sh_new = h1h[:, (t + 1) * 2 * nb:(t + 2) * 2 * nb]
                    nc.vector.tensor_scalar_mul(sh_new, tt[:], 0.5)
                    xin = sh_new

        NCH = min(512, TB)
        h1h4 = h1h[:].rearrange("p (t c b) -> p t c b", c=2, b=nb)
        with nc.named_scope("logits"):
            for ch in range(TB // NCH):
                tpc = NCH // nb
                lp = (PB_LG if ch % 2 == 0 else PB_CP)[:, 0:NCH]
                for kc in range(2):
                    rhs = h1h4[:, 1 + ch * tpc: 1 + (ch + 1) * tpc, kc, :]
                    nc.tensor.matmul(lp, owt[:, kc * V:(kc + 1) * V], rhs,
                                     start=(kc == 0), stop=(kc == 1))
                ex16 = work.tile([128, NCH], BF16, tag="ex16", name="ex16")
                nc.scalar.activation(ex16[:], lp, AF.Exp, bias=outb[:, 0:1])
                zr = (PB_SC if ch % 2 == 0 else PB_RZ0)[0:1, 0:NCH]
                nc.tensor.matmul(zr, onescol[:, 0:1], ex16[:], start=True, stop=True)
                lnz = work.tile([1, NCH], F16, tag="lnz", name="lnz")
                nc.scalar.activation(lnz[:], zr, AF.Ln)
                bcl = (PB_S1 if ch % 2 == 0 else PB_HN)[:, 0:NCH]
                nc.tensor.matmul(bcl, onesrow16[:, :], lnz[:], start=True, stop=True)
                bcl_s = work.tile([128, NCH], F16, tag="bcl_s", name="bcl_s")
                nc.scalar.copy(bcl_s[:], bcl)
                lout = work.tile([128, NCH], F32, tag="lout", name="lout")
                nc.vector.scalar_tensor_tensor(lout[:], lp, outb[:, 0:1], bcl_s[:],
                                               OP.add, OP.subtract)
                nc.sync.dma_start(d_logp[:, ch * NCH:(ch + 1) * NCH], lout[:])

        exph4 = exph[:].rearrange("p (t c b) -> p t c b", c=2, b=nb)
        with nc.named_scope("attn"):
            for ch in range(TB // NCH):
                tpc = NCH // nb
                bca = (PB_RZ1 if ch % 2 == 0 else PB_BZ)[:, 0:NCH]
                nc.tensor.matmul(bca, onesrow[:, :],
                                 rzh[0:1, ch * NCH:(ch + 1) * NCH],
                                 start=True, stop=True)
                for lc in range(2):
                    an = work.tile([128, NCH], F32, tag="an", name="an")
                    nc.vector.tensor_tensor(
                        an[:], exph4[:, ch * tpc:(ch + 1) * tpc, lc, :], bca, OP.mult)
                    nc.sync.dma_start(
                        d_attn[lc * 128:(lc + 1) * 128, ch * NCH:(ch + 1) * NCH], an[:])

    nc.compile()
    return nc


def _host_prep(inputs, core, Tn=T, nb=NB):
    x = np.asarray(inputs["x"])[:, :, 0, :]
    y = np.asarray(inputs["y"]).astype(np.int64)
    emb = np.asarray(inputs["emb"], np.float32)
    attn_w = np.asarray(inputs["attn_w"], np.float32)
    attn_b = np.asarray(inputs["attn_b"], np.float32)
    comb_w = np.asarray(inputs["comb_w"], np.float32)
    comb_b = np.asarray(inputs["comb_b"], np.float32)
    gwih = np.asarray(inputs["gru_w_ih"], np.float32)
    gwhh = np.asarray(inputs["gru_w_hh"], np.float32)
    gbih = np.asarray(inputs["gru_b_ih"], np.float32)
    gbhh = np.asarray(inputs["gru_b_hh"], np.float32)
    out_w = np.asarray(inputs["out_w"], np.float32)
    out_b = np.asarray(inputs["out_b"], np.float32)
    assert np.all(gbih == 0) and np.all(gbhh == 0), "nonzero GRU biases unsupported"

    sl = slice(core * nb, (core + 1) * nb)
    tokens = np.concatenate([np.zeros((B, 1), y.dtype), y[:, :-1]], axis=1)[sl]
    xb = x[sl]
    A1, A2 = attn_w[:, :H], attn_w[:, H:]
    C1, C2 = comb_w[:, :H], comb_w[:, H:]

    ea1_tab = np.exp(A1 @ emb.T + attn_b[:, None]).astype(np.float32)
    c2e_tab = (C2 @ emb.T + comb_b[:, None]).astype(np.float32)

    tb = Tn * nb
    ea1_g = ea1_tab[:, tokens.T[:Tn]]
    ea1_r = ea1_g.reshape(2, 128, Tn, nb).transpose(1, 2, 0, 3).reshape(128, 2 * tb)
    c2e_g = c2e_tab[:, tokens.T[:Tn]]
    c2e_r = c2e_g.reshape(2, 128, Tn, nb).transpose(1, 2, 0, 3).reshape(128, 2 * tb)

    def lhs_blocks(Wt, n_kc, n_mc):
        K, M = Wt.shape
        assert K == n_kc * 128 and M == n_mc * 128
        a = Wt.reshape(n_kc, 128, n_mc, 128).transpose(1, 0, 2, 3)
        return a.reshape(128, n_kc * n_mc * 128)

    c1t_rhs = C1.T.reshape(2, 128, 256).transpose(1, 0, 2).reshape(128, 2 * H)
    a2t_r = lhs_blocks(A2.T, 2, 2)

    wrz_list, win_list, whn_list = [], [], []
    for lyr in range(LAYERS):
        Wi, Wh = gwih[lyr], gwhh[lyr]
        lhs_rz = np.vstack([Wi[0:2 * H, :].T, Wh[0:2 * H, :].T])
        wrz_list.append(lhs_blocks(lhs_rz, 4, 4))
        win_list.append(lhs_blocks(Wi[2 * H:3 * H, :].T, 2, 2))
        whn_list.append(lhs_blocks(0.5 * Wh[2 * H:3 * H, :].T, 2, 2))
    wrz_r = np.concatenate(wrz_list, axis=1)
    win_r = np.concatenate(win_list, axis=1)
    whn_r = np.concatenate(whn_list, axis=1)
    owt_r = lhs_blocks(out_w.T, 2, 1)

    return {
        "x_r": np.ascontiguousarray(xb.reshape(nb, 2, 128, L)).astype(BF),
        "c1t": c1t_rhs.astype(BF),
        "a2t": a2t_r.astype(BF),
        "ea1": ea1_r.astype(BF),
        "c2e": c2e_r.astype(BF),
        "wrz": wrz_r.astype(BF),
        "win": win_r.astype(BF),
        "whn": whn_r.astype(BF),
        "owt": owt_r.astype(BF),
        "outb": out_b.reshape(128, 1).astype(np.float32),
        "onescol": np.ones((128, 1), BF),
        "onesrow": np.ones((1, 128), BF),
    }


def _host_post(res, Tn=T, nb=NB):
    lp = res["out_logp"].reshape(V, Tn, nb)
    outs = np.ascontiguousarray(lp.transpose(2, 0, 1))
    at = res["out_attn"].reshape(2, 128, Tn, nb)
    attns = np.ascontiguousarray(at.transpose(3, 2, 0, 1).reshape(nb, Tn, L))
    return outs, attns


class _Runner:
    def __init__(self, nc, n_cores):
        install_neuronx_cc_hook()
        self.nc = nc
        self.n_cores = n_cores
        in_names, out_names, out_avals, zero_outs = [], [], [], []
        partition_name = nc.partition_id_tensor.name if nc.partition_id_tensor else None
        for alloc in nc.m.functions[0].allocations:
            if not isinstance(alloc, mybir.MemoryLocationSet):
                continue
            name = alloc.memorylocations[0].name
            if alloc.kind == "ExternalInput":
                if name != partition_name:
                    in_names.append(name)
            elif alloc.kind == "ExternalOutput":
                shape = tuple(alloc.tensor_shape)
                dtype = mybir.dt.np(alloc.dtype)
                out_names.append(name)
                out_avals.append(jax.core.ShapedArray(shape, dtype))
                zero_outs.append(np.zeros(shape, dtype))
        self.in_names, self.out_names = in_names, out_names
        self.out_avals, self.zero_outs = out_avals, zero_outs
        n_params, n_outs = len(in_names), len(out_avals)
        all_in_names = list(in_names) + list(out_names)
        if partition_name is not None:
            all_in_names.append(partition_name)

        def _body(*args):
            operands = list(args)
            if partition_name is not None:
                operands.append(partition_id_tensor())
            outs = _bass_exec_p.bind(
                *operands,
                out_avals=tuple(out_avals),
                in_names=tuple(all_in_names),
                out_names=tuple(out_names),
                lowering_input_output_aliases=(),
                sim_require_finite=True,
                sim_require_nnan=True,
                nc=nc,
            )
            return tuple(outs)

        devices = jax.devices()[:n_cores]
        mesh = Mesh(np.asarray(devices), ("core",))
        in_specs = (PartitionSpec("core"),) * (n_params + n_outs)
        out_specs = (PartitionSpec("core"),) * len(out_names)
        self.fn = jax.jit(
            shard_map(_body, mesh=mesh, in_specs=in_specs, out_specs=out_specs,
                      check_rep=False),
            keep_unused=True)

    def run(self, in_maps):
        concat_in = [np.concatenate([np.asarray(m[n]) for m in in_maps], axis=0)
                     for n in self.in_names]
        concat_zero = [np.zeros((self.n_cores * z.shape[0], *z.shape[1:]), z.dtype)
                       for z in self.zero_outs]
        outs = self.fn(*concat_in, *concat_zero)
        res = []
        for c in range(self.n_cores):
            d = {}
            for i, n in enumerate(self.out_names):
                a = np.asarray(outs[i])
                d[n] = a.reshape(self.n_cores, *self.out_avals[i].shape)[c]
            res.append(d)
        return res


_CACHE = {}


def _get_runner():
    if "r" not in _CACHE:
        nc = _build_nc()
        _CACHE["r"] = _Runner(nc, N_CORES)
    return _CACHE["r"]


def kernel(**inputs):
    runner = _get_runner()
    in_maps = [_host_prep(inputs, c) for c in range(N_CORES)]
    res = runner.run(in_maps)
    outs = np.zeros((B, V, T), np.float32)
    attns = np.zeros((B, T, L), np.float32)
    for c in range(N_CORES):
        o, a = _host_post(res[c])
        outs[c * NB:(c + 1) * NB] = o
        attns[c * NB:(c + 1) * NB] = a
    return outs, attns
